# revision 29
# baseline (speedup 1.0000x reference)
import sys

sys.path.insert(0, "/opt/trn_rl_repo")
import numpy as np

DIM = 1024
HEADS = 16
HD = 64
HID = 4096
EPS = 1e-5
NQ = 512          # queries per core
NK = 2048
P = 128

SQ = 64.0         # wq scale (fp8 range)
SK = 64.0         # wk scale
SV = 64.0         # wv scale
SO = 64.0         # wo scale
ESC = 0.125 / (SQ * SK)   # exp scale: scores/temp with fp8 scales folded out
SCH_A = 12102203.161561486 * ESC   # 2^23/ln2, folded with ESC
SCH_B = 1064986823.0               # 127*2^23 - 366393 (min-RMS bias)
OSC = 1.0 / (SV * SO)     # out-proj descale (ctx8 = SV*ctx_normed, wo8 = SO*wo)

_CACHE = {}


def _build():
    import concourse.bacc as bacc
    import concourse.tile as tile
    from concourse import mybir
    from concourse.masks import make_identity
    from contextlib import ExitStack

    F32 = mybir.dt.float32
    BF16 = mybir.dt.bfloat16
    F8 = mybir.dt.float8e4
    AF = mybir.ActivationFunctionType
    DR = mybir.MatmulPerfMode.DoubleRow

    nc = bacc.Bacc(None, target_bir_lowering=False, debug=False)

    tgt = nc.declare_dram_parameter("tgt", [NQ, DIM], F32, isOutput=False)
    # emb8[p, kc, ic, n] = fp8(emb[n, 256*kc + 128*ic + p])
    emb8 = nc.declare_dram_parameter("emb8", [P, 4, 2, NK], F8, isOutput=False)
    # wq8[p, g, io, kc, ic, c(=32h'+j)] = SQ*wq_e[256kc+128ic+p, 256g+64h'+32io+j]
    wq8 = nc.declare_dram_parameter("wq8", [P, 4, 2, 4, 2, P], F8, isOutput=False)
    wk8 = nc.declare_dram_parameter("wk8", [P, 4, 2, 4, 2, P], F8, isOutput=False)
    # wv8[p, g, kc, ic, j(=64hq+d)] = SV*wv[256kc+128ic+p, 256g+j]
    wv8 = nc.declare_dram_parameter("wv8", [P, 4, 4, 2, 256], F8, isOutput=False)
    # wo8[p(0:64), g8, ic, m] = SO*wo[64*(2*g8+ic)+p, m]
    wo8 = nc.declare_dram_parameter("wo8", [64, 8, 2, DIM], F8, isOutput=False)
    # w1b[hg][p][kc, hcol] = bf16(w1_e[128kc+p, 512hg+hcol])
    w1b = nc.declare_dram_parameter("w1b", [8, P, 8, 512], BF16, isOutput=False)
    # w2b[hm][p][m] = bf16(w2[128hm+p, m])
    w2b = nc.declare_dram_parameter("w2b", [HID // P, P, DIM], BF16, isOutput=False)
    # biasf: [128, 48]: cols 0:8 SQ*bq_e by (g,io), 8:16 SK*bk, 16:48 b1_e by hm
    biasf = nc.declare_dram_parameter("biasf", [P, 48], F32, isOutput=False)
    bvb = nc.declare_dram_parameter("bvb", [2 * DIM], F8, isOutput=False)  # SV*bv, dup x2
    bob = nc.declare_dram_parameter("bob", [DIM], BF16, isOutput=False)   # bo
    b2b = nc.declare_dram_parameter("b2b", [DIM], BF16, isOutput=False)   # b2
    out = nc.declare_dram_parameter("out", [NQ, DIM], BF16, isOutput=True)

    def bcast_ap(vec, n):
        import concourse.bass as bass
        return bass.AP(tensor=vec.tensor, offset=vec.offset, ap=[[0, P], [1, n]])

    with tile.TileContext(nc) as tc, ExitStack() as S:
        const = S.enter_context(tc.tile_pool(name="const", bufs=1))

        identf = const.tile([P, P], F32)
        make_identity(nc, identf)
        identb = const.tile([P, P], BF16)
        nc.scalar.activation(identb[:], identf[:], AF.Copy)
        eps_t = const.tile([P, 1], F32)
        nc.vector.memset(eps_t[:], EPS)

        bp = const.tile([P, 48], F32)
        bvb_b = const.tile([P, 2 * DIM], F8)
        bob_b = const.tile([P, DIM], BF16)
        b2b_b = const.tile([P, DIM], BF16)

        # ---- persistent SBUF ----
        perm = S.enter_context(tc.tile_pool(name="perm", bufs=1))
        eT8 = perm.tile([P, 4, 2, NK], F8)          # emb, feature-major fp8
        wk_sb = perm.tile([P, 4, 2, 4, 2, P], F8)
        tgt_t = [perm.tile([P, DIM], F32, name=f"tgt{t}") for t in range(4)]
        nc.sync.dma_start(out=wk_sb[:, 0], in_=wk8[:, 0])
        for kc in range(4):
            nc.sync.dma_start(out=eT8[:, kc, :, :], in_=emb8[:, kc, :, :])
        for g in range(1, 4):
            nc.sync.dma_start(out=wk_sb[:, g], in_=wk8[:, g])
        for t in range(4):
            nc.sync.dma_start(out=tgt_t[t][:], in_=tgt[t * P:(t + 1) * P, :])
        nc.sync.dma_start(out=bp[:], in_=biasf[:, :])
        wv_sb = perm.tile([P, 4, 4, 2, 256], F8)
        nc.sync.dma_start(out=wv_sb[:], in_=wv8[:, :, :, :, :])
        wo_sb = perm.tile([64, 8, 2, DIM], F8)
        nc.sync.dma_start(out=wo_sb[:], in_=wo8[:, :, :, :])
        nc.sync.dma_start(out=bvb_b[:], in_=bcast_ap(bvb[:], 2 * DIM))
        nc.sync.dma_start(out=bob_b[:], in_=bcast_ap(bob[:], DIM))
        nc.sync.dma_start(out=b2b_b[:], in_=bcast_ap(b2b[:], DIM))

        K8g = [perm.tile([P, 2, NK], F8, name=f"K8_{g}") for g in range(4)]
        Q8g = [perm.tile([P, 2, NQ], F8, name=f"Q8_{g}") for g in range(4)]
        # head 3 of each group sits at partition base 96, which matmul APs
        # can't address -> DMA-shift its 32 partitions down to base 0
        K8h3 = [perm.tile([32, 2, NK], F8, name=f"K8h3_{g}") for g in range(4)]
        Q8h3 = [perm.tile([32, 2, NQ], F8, name=f"Q8h3_{g}") for g in range(4)]
        lnT8 = [perm.tile([P, 2, NQ], F8, name=f"lnT8_{k}") for k in range(4)]
        ctx8 = [perm.tile([64, 2, NQ], F8, name=f"ctx8_{g8}") for g8 in range(8)]
        tgt2 = [perm.tile([P, DIM], F32, name=f"tgt2_{t}") for t in range(4)]
        ln2T = [[perm.tile([P, P], BF16, name=f"ln2T_{k}_{t}") for t in range(4)]
                for k in range(8)]
        h1T = [perm.tile([P, NQ], BF16, name=f"h1T_{m}") for m in range(HID // P)]

        def layer_norm_rows(x, y, pool, nm):
            # y = (x - mean)/std rowwise over 1024, y may be bf16
            st = pool.tile([P, 2, nc.vector.BN_STATS_DIM], F32, name=f"st{nm}", tag="st")
            for sg in range(2):
                nc.vector.bn_stats(out=st[:, sg, :], in_=x[:, sg * 512:(sg + 1) * 512])
            mv = pool.tile([P, nc.vector.BN_AGGR_DIM], F32, name=f"mv{nm}", tag="mv")
            nc.vector.bn_aggr(out=mv[:], in_=st[:])
            rstd = pool.tile([P, 1], F32, name=f"rstd{nm}", tag="rstd")
            nc.scalar.activation(out=rstd[:], in_=mv[:, 1:2], func=AF.Sqrt,
                                 bias=eps_t[:], scale=1.0)
            nc.vector.reciprocal(out=rstd[:], in_=rstd[:])
            nb = pool.tile([P, 1], F32, name=f"nb{nm}", tag="nb")
            nc.vector.tensor_scalar(out=nb[:], in0=mv[:, 0:1], scalar1=rstd[:],
                                    scalar2=-1.0, op0=mybir.AluOpType.mult,
                                    op1=mybir.AluOpType.mult)
            nc.scalar.activation(out=y[:], in_=x[:], func=AF.Identity,
                                 bias=nb[:], scale=rstd[:])

        # ---------- Phase 1 + 2: projections and attention, software-pipelined ----------
        kq_cm = tc.tile_pool(name="kq_ps", bufs=2, space="PSUM")
        kq_ps = kq_cm.__enter__()

        ln1_cm = tc.tile_pool(name="ln1", bufs=1)
        ln1 = ln1_cm.__enter__()

        wq_cm = tc.tile_pool(name="wqp", bufs=1)
        wqp = wq_cm.__enter__()
        wq_sb = wqp.tile([P, 4, 2, 4, 2, P], F8)
        nc.sync.dma_start(out=wq_sb[:], in_=wq8[:, :, :, :, :, :])

        def emit_kproj(g, ios=(0, 1)):
            # kproj group g -> K8g[g] (fp8 DoubleRow, contraction 256)
            # group 0 converts on Act (pre-phase); later groups on DVE
            for io in ios:
                for nck in range(4):
                    ps = kq_ps.tile([P, NQ], F32, name="kps", tag="kq")
                    for kc in range(4):
                        nc.tensor.matmul(
                            ps[:], wk_sb[:, g, io, kc, :, :],
                            eT8[:, kc, :, nck * NQ:(nck + 1) * NQ],
                            start=(kc == 0), stop=(kc == 3), perf_mode=DR)
                    if g == 0:
                        nc.scalar.activation(
                            K8g[g][:, io, nck * NQ:(nck + 1) * NQ], ps[:],
                            AF.Identity, bias=bp[:, 8 + 2 * g + io:9 + 2 * g + io])
                    else:
                        nc.vector.tensor_scalar_add(
                            K8g[g][:, io, nck * NQ:(nck + 1) * NQ], ps[:],
                            bp[:, 8 + 2 * g + io:9 + 2 * g + io])
            if ios[-1] == 1:
                nc.sync.dma_start(out=K8h3[g][:], in_=K8g[g][96:128, :, :])

        emit_kproj(0)
        with tc.tile_pool(name="lnw", bufs=4) as lnw, \
             tc.tile_pool(name="tp_ps", bufs=4, space="PSUM") as tp_ps:
            # LN1 rows (bf16) on DVE while kproj g0 runs on PE
            ln1r = [ln1.tile([P, DIM], BF16, name=f"ln1r{t}") for t in range(4)]
            for t in range(4):
                layer_norm_rows(tgt_t[t], ln1r[t], lnw, f"a{t}")
            for t in range(4):
                for kc in range(4):
                    for ic in range(2):
                        pt = tp_ps.tile([P, P], BF16, name="pt", tag="tp")
                        f0 = 256 * kc + 128 * ic
                        nc.tensor.transpose(pt[:], ln1r[t][:, f0:f0 + P], identb[:])
                        nc.vector.tensor_copy(lnT8[kc][:, ic, t * P:(t + 1) * P], pt[:])
            # qproj -> Q8 (converts on Act)
            for g in range(4):
                for io in range(2):
                    ps = kq_ps.tile([P, NQ], F32, name="qps", tag="kq")
                    for kc in range(4):
                        nc.tensor.matmul(ps[:], wq_sb[:, g, io, kc, :, :], lnT8[kc][:],
                                         start=(kc == 0), stop=(kc == 3), perf_mode=DR)
                    nc.scalar.activation(Q8g[g][:, io, :], ps[:], AF.Identity,
                                         bias=bp[:, 2 * g + io:2 * g + io + 1])
                nc.sync.dma_start(out=Q8h3[g][:], in_=Q8g[g][96:128, :, :])
        wq_cm.__exit__(None, None, None)
        ln1_cm.__exit__(None, None, None)

        # ---------- attention (fp8 DoubleRow), kproj/vproj g>=1 interleaved ----------
        with tc.tile_pool(name="v8p", bufs=2) as v8p, \
             tc.tile_pool(name="exp8", bufs=3) as exp8, \
             tc.tile_pool(name="sm", bufs=2) as sm, \
             tc.tile_pool(name="scp", bufs=2, space="PSUM") as scp, \
             tc.tile_pool(name="cxp", bufs=2, space="PSUM") as cxp:

            v8 = {}

            def emit_vproj(g, ts):
                # v8[g][t]: [128, 2(ic=kvt parity), 4(hq), 68] fp8; col 64 = ones
                if g not in v8:
                    v8[g] = [v8p.tile([P, 2, 4, 68], F8, name=f"v8_{g}_{t}", tag=f"v{t}")
                             for t in range(8)]
                for t in ts:
                    ps = kq_ps.tile([P, NQ], F32, name="vp", tag="kq")
                    for half in range(2):   # kvt = 2t + half
                        kvt = 2 * t + half
                        for kc in range(4):
                            nc.tensor.matmul(
                                ps[:, half * 256:(half + 1) * 256],
                                eT8[:, kc, :, kvt * P:(kvt + 1) * P],
                                wv_sb[:, g, kc, :, :],
                                start=(kc == 0), stop=(kc == 3), perf_mode=DR)
                    nc.vector.tensor_tensor(
                        out=v8[g][t][:, :, :, 0:64],
                        in0=ps[:].rearrange("p (i h d) -> p i h d", i=2, h=4),
                        in1=bvb_b[:, 512 * g:512 * (g + 1)].rearrange(
                            "p (i h d) -> p i h d", i=2, h=4),
                        op=mybir.AluOpType.add)
                    nc.gpsimd.memset(v8[g][t][:, :, :, 64:65], 1.0)

            emit_vproj(0, range(8))
            for g in range(4):
                for h in range(4):
                    head = 4 * g + h
                    g8, ih = divmod(head, 2)
                    cps = cxp.tile([P, NQ], F32, name="cps", tag="cps")
                    if h < 3:
                        Ksrc, Qsrc, pb = K8g[g], Q8g[g], 32 * h
                    else:
                        Ksrc, Qsrc, pb = K8h3[g], Q8h3[g], 0
                    for t in range(8):
                        sc = scp.tile([P, 2 * NQ], F32, name="sc", tag="sc")
                        for half in range(2):
                            kvt = 2 * t + half
                            nc.tensor.matmul(
                                sc[:, half * NQ:(half + 1) * NQ],
                                Ksrc[pb:pb + 32, :, kvt * P:(kvt + 1) * P],
                                Qsrc[pb:pb + 32, :, :],
                                start=True, stop=True, perf_mode=DR)
                        ex = exp8.tile([P, 2 * NQ], F8, name="ex", tag="ex")
                        nc.scalar.activation(ex[:], sc[:], AF.Exp, scale=ESC)
                        nc.tensor.matmul(
                            cps[0:65, :], v8[g][t][:, :, h, 0:65],
                            ex[:].rearrange("p (i n) -> p i n", i=2),
                            start=(t == 0), stop=(t == 7), perf_mode=DR)
                    # interleave next quarter's vproj/kproj behind this quarter
                    if g < 3:
                        if h == 0:
                            emit_vproj(g + 1, range(0, 4))
                        elif h == 1:
                            emit_vproj(g + 1, range(4, 8))
                        elif h == 2:
                            emit_kproj(g + 1, (0,))
                        else:
                            emit_kproj(g + 1, (1,))
                    # softmax tail: normalize by denominator (row 64)
                    rl = sm.tile([P, NQ], F32, name="rl", tag="rl")
                    nc.vector.reciprocal(out=rl[64:65, :], in_=cps[64:65, :])
                    rl0 = sm.tile([1, NQ], F32, name="rl0", tag="rl0")
                    nc.sync.dma_start(out=rl0[0:1, :], in_=rl[64:65, :])
                    bcs = sm.tile([64, NQ], F32, name="bcs", tag="bcs")
                    nc.gpsimd.partition_broadcast(bcs[:], rl0[0:1, :], channels=64)
                    nc.vector.tensor_tensor(
                        out=ctx8[g8][0:64, ih, :], in0=cps[0:64, :], in1=bcs[:],
                        op=mybir.AluOpType.mult)

        kq_cm.__exit__(None, None, None)

        # ---------- Phase 3: out-proj + residual -> tgt2; LN2 -> ln2T ----------
        ln2_cm = tc.tile_pool(name="ln2", bufs=1)
        ln2 = ln2_cm.__enter__()
        ln2r = [ln2.tile([P, DIM], BF16, name=f"ln2r{t}") for t in range(4)]
        with tc.tile_pool(name="ln2w", bufs=4) as ln2w, \
             tc.tile_pool(name="o_ps", bufs=2, space="PSUM") as o_ps, \
             tc.tile_pool(name="l2_ps", bufs=4, space="PSUM") as l2_ps:
            for t in range(4):
                # fold +bo into residual before the STT add
                nc.gpsimd.tensor_tensor(out=tgt_t[t][:], in0=tgt_t[t][:],
                                        in1=bob_b[:], op=mybir.AluOpType.add)
                ps = o_ps.tile([P, DIM], F32, name="ops", tag="op")
                for g8 in range(8):
                    for mh in range(2):
                        nc.tensor.matmul(ps[:, mh * 512:(mh + 1) * 512],
                                         ctx8[g8][:, :, t * P:(t + 1) * P],
                                         wo_sb[:, g8, :, mh * 512:(mh + 1) * 512],
                                         start=(g8 == 0), stop=(g8 == 7), perf_mode=DR)
                nc.vector.scalar_tensor_tensor(
                    out=tgt2[t][:], in0=ps[:], scalar=OSC, in1=tgt_t[t][:],
                    op0=mybir.AluOpType.mult, op1=mybir.AluOpType.add)
                layer_norm_rows(tgt2[t], ln2r[t], ln2w, f"b{t}")
                # b2 folded into tgt2 AFTER stats are taken (fc2 residual)
                nc.gpsimd.tensor_tensor(out=tgt2[t][:], in0=tgt2[t][:],
                                        in1=b2b_b[:], op=mybir.AluOpType.add)
                for k in range(8):
                    pt = l2_ps.tile([P, P], BF16, name="l2pt", tag="l2tp")
                    nc.tensor.transpose(pt[:], ln2r[t][:, k * P:(k + 1) * P], identb[:])
                    nc.scalar.activation(ln2T[k][t][:], pt[:], AF.Copy)
        ln2_cm.__exit__(None, None, None)

        # ---------- Phase 4: fc1 (bf16) + gelu ----------
        with tc.tile_pool(name="w1s", bufs=2) as w1s, \
             tc.tile_pool(name="f1_ps", bufs=2, space="PSUM") as f1_ps:
            for hg in range(8):
                wt = w1s.tile([P, 8, 512], BF16, name="w1t", tag="w1")
                nc.sync.dma_start(out=wt[:], in_=w1b[hg])
                for sub in range(2):
                    pss = [f1_ps.tile([P, NQ], F32, name=f"f1p{j}", tag=f"f1_{j}")
                           for j in range(2)]
                    for qt in range(4):
                        for kc in range(8):
                            for j in range(2):
                                hc = 2 * sub + j
                                nc.tensor.matmul(
                                    pss[j][:, qt * P:(qt + 1) * P],
                                    wt[:, kc, hc * P:(hc + 1) * P],
                                    ln2T[kc][qt][:], start=(kc == 0), stop=(kc == 7))
                    for j in range(2):
                        hm = hg * 4 + 2 * sub + j
                        nc.scalar.activation(h1T[hm][:], pss[j][:], AF.Gelu,
                                             bias=bp[:, 16 + hm:17 + hm])

        # ---------- Phase 5: fc2 (bf16) + residual + store ----------
        with tc.tile_pool(name="w2s", bufs=4) as w2s, \
             tc.tile_pool(name="f2_ps", bufs=1, space="PSUM") as f2_ps:
            pss = [f2_ps.tile([P, DIM], F32, name=f"f2p{t}", tag=f"f2_{t}")
                   for t in range(4)]
            for hm in range(HID // P):
                wt = w2s.tile([P, DIM], BF16, name="w2t", tag="w2")
                nc.sync.dma_start(out=wt[:], in_=w2b[hm])
                for t in range(4):
                    for mh in range(2):
                        nc.tensor.matmul(pss[t][:, mh * 512:(mh + 1) * 512],
                                         h1T[hm][:, t * P:(t + 1) * P],
                                         wt[:, mh * 512:(mh + 1) * 512],
                                         start=(hm == 0), stop=(hm == HID // P - 1))
            for t in range(4):
                ob = w2s.tile([P, DIM], BF16, name=f"ob{t}", tag="ob")
                nc.vector.tensor_tensor(out=ob[:], in0=pss[t][:],
                                        in1=tgt2[t][:], op=mybir.AluOpType.add)
                nc.sync.dma_start(out=out[t * P:(t + 1) * P, :], in_=ob[:])

    nc.compile()
    return nc


def _get_nc():
    if "nc" not in _CACHE:
        _CACHE["nc"] = _build()
    return _CACHE["nc"]


def kernel(tgt, emb_motion, ln_g, ln_b, wq, bq, wk, bk, wv, bv, wo, bo, w1, b1, w2, b2):
    import ml_dtypes
    from concourse.bass_utils import run_bass_kernel_spmd

    nc = _get_nc()
    f = np.ascontiguousarray
    a32 = lambda x: np.asarray(x, np.float32)
    FP8 = ml_dtypes.float8_e4m3
    BF = ml_dtypes.bfloat16

    def q8(x):
        return np.clip(x, -440.0, 440.0).astype(FP8)

    # fold LN affine into wq/w1 (exact: (xh*g+b)@W = xh@(g*W) + (b@W))
    g32, b32 = a32(ln_g), a32(ln_b)
    wq_e = a32(wq) * g32[:, None]
    bq_e = a32(bq) + b32 @ a32(wq)
    w1_e = a32(w1) * g32[:, None]
    b1_e = a32(b1) + b32 @ a32(w1)

    # wq8/wk8: [p, g, io, kc, ic, c] = S*W[256kc+128ic+p, 256g+64h'+32io+j], c=32h'+j
    def pack_qk(W, S):
        A = (a32(W) * S).reshape(4, 2, 128, 4, 4, 2, 32)  # [kc, ic, p, g, h', io, j]
        return q8(f(A.transpose(2, 3, 5, 0, 1, 4, 6).reshape(128, 4, 2, 4, 2, 128)))

    wq8 = pack_qk(wq_e, SQ)
    wk8 = pack_qk(wk, SK)
    # wv8: [p, g, kc, ic, j] = SV*wv[256kc+128ic+p, 256g+j]
    A = (a32(wv) * SV).reshape(4, 2, 128, 4, 256)          # [kc, ic, p, g, j]
    wv8 = q8(f(A.transpose(2, 3, 0, 1, 4)))
    # wo8: [p, g8, ic, m] = SO*wo[64*(2g8+ic)+p, m]
    A = (a32(wo) * SO).reshape(8, 2, 64, 1024)             # [g8, ic, p, m]
    wo8 = q8(f(A.transpose(2, 0, 1, 3)))
    # w1b: [hg, p, kc, hcol] ; w2b: [hm, p, m]
    A = w1_e.reshape(8, 128, 8, 512)                       # [kc, p, hg, hcol]
    w1bh = f(A.transpose(2, 1, 0, 3)).astype(BF)
    w2bh = f(a32(w2).reshape(32, 128, 1024)).astype(BF)

    # biasf [128, 48]
    biasf = np.zeros((128, 48), np.float32)
    bq_s = (SQ * bq_e).reshape(4, 4, 2, 32)                # [g, h', io, j]
    bk_s = (SK * a32(bk)).reshape(4, 4, 2, 32)
    for g in range(4):
        for io in range(2):
            biasf[:, 2 * g + io] = bq_s[g, :, io, :].reshape(128)
            biasf[:, 8 + 2 * g + io] = bk_s[g, :, io, :].reshape(128)
    biasf[:, 16:48] = b1_e.reshape(32, 128).T

    bvb = q8(np.tile((SV * a32(bv)).reshape(4, 1, 256), (1, 2, 1)).reshape(2048))
    bob = a32(bo).astype(BF)
    b2b = a32(b2).astype(BF)

    B = tgt.shape[0]
    emb8_by_b = {}
    for b in range(B):
        # emb8[p, kc, ic, n] = fp8(emb[n, 256kc+128ic+p])
        E = a32(emb_motion[b]).T.reshape(4, 2, 128, NK)    # [kc, ic, p, n]
        emb8_by_b[b] = q8(f(E.transpose(2, 0, 1, 3)))

    in_maps = []
    for c in range(8):
        b, h = divmod(c, 2)
        in_maps.append({
            "tgt": f(a32(tgt[b, h * NQ:(h + 1) * NQ])),
            "emb8": emb8_by_b[b],
            "wq8": wq8, "wk8": wk8, "wv8": wv8, "wo8": wo8,
            "w1b": w1bh, "w2b": w2bh,
            "biasf": biasf, "bvb": bvb, "bob": bob, "b2b": b2b,
        })
    r = run_bass_kernel_spmd(nc, in_maps, list(range(8)))
    res = np.empty((B, 1024, DIM), np.float32)
    for c in range(8):
        b, h = divmod(c, 2)
        res[b, h * NQ:(h + 1) * NQ] = np.asarray(r.results[c]["out"], np.float32)
    return res
